# revision 4
# baseline (speedup 1.0000x reference)
"""Depthwise 7x7 conv (stride 1, pad 3) on 8 NeuronCores via Bass.

Strategy: channel-sharded SPMD (48 channels/core).  Per channel, conv along H
is a banded matmul on TensorE (stationary = banded filter matrix G, moving =
X rows); the 7 kw taps accumulate in PSUM via free-dim-shifted rhs slices.
Inputs stream as fp16 (half DMA bytes, full PE rate); outputs store as fp16
and are upcast on host.

Host pre-gathers X into per-channel [128, 4*512] window blocks and packs G
4-channels-per-group so every DMA moves >=4KB contiguous per partition.
H tiling: 4 full 122-row tiles per channel + a 24-row runt; runts of 4
channels pack onto diagonal 32x32 PE tiles and run concurrently (kw-outer
issue order).  Loads go on the SP HWDGE ring, stores on the ACT ring.
"""

import os as _os

import numpy as np

import concourse.bacc as bacc
import concourse.mybir as mybir
import concourse.tile as tile
from concourse.bass_utils import run_bass_kernel_spmd

C, H, W_DIM = 384, 512, 512
KH = KW = 7
PAD = 3
N_CORES = 8
CPC = C // N_CORES   # 48 channels per core
NG = CPC // 4        # 12 groups of 4 channels per core

GW = 125             # banded-matrix width per kw tap
KWGW = KW * GW       # 875 columns per channel in G
MT = 122             # output rows per full tile
NFULL = 4            # full tiles per channel
MR = H - NFULL * MT  # runt output rows (24)
KR = MR + PAD        # runt contraction rows (27)
HP = 520             # padded rows per channel (3 zero top + 512 + 5 zero tail)

N_XBUF = int(_os.environ.get("N_XBUF", "10"))
N_OBUF = int(_os.environ.get("N_OBUF", "8"))
WARMUP_MM = int(_os.environ.get("WARMUP_MM", "20"))

F32 = mybir.dt.float32
F16 = mybir.dt.float16
NP_IN = np.float16

KWS = [PAD] + [k for k in range(KW) if k != PAD]  # kw=3 first: full-width start


def _emit(nc, x_pool, g_pool, ps_pool, o_pool,
          x_dram, xr_dram, g_dram, y_dram, yr_dram):
    w = W_DIM
    x_ts = [
        x_pool.tile([128, NFULL * w], F16, tag=f"x{i}", name=f"x{i}")
        for i in range(N_XBUF)
    ]
    xr_ts = [
        x_pool.tile([128, w], F16, tag=f"xr{i}", name=f"xr{i}") for i in range(3)
    ]
    o_ts = [
        o_pool.tile([128, NFULL * w], F16, tag=f"o{i}", name=f"o{i}")
        for i in range(N_OBUF)
    ]
    or_ts = [
        o_pool.tile([128, w], F16, tag=f"or{i}", name=f"or{i}") for i in range(3)
    ]

    if WARMUP_MM:
        # Keep PE busy (and HAM un-throttled) while the first loads land.
        g_w = o_pool.tile([128, MT], F16, tag="gw", name="g_w")
        x_w = o_pool.tile([128, w], F16, tag="xw0", name="x_w")
        nc.vector.memset(g_w[:, :], 0.0)
        nc.vector.memset(x_w[:, :], 0.0)
        ps_w = ps_pool.tile([128, w], F32, tag="ps", name="ps_w")
        for i in range(WARMUP_MM):
            nc.tensor.matmul(
                ps_w[:MT, :], g_w[:, :MT], x_w[:, :],
                start=(i == 0), stop=(i == WARMUP_MM - 1),
            )
        nc.vector.tensor_copy(x_w[:MT, :], ps_w[:MT, :])

    for g in range(NG):
        g_t = g_pool.tile([128, 4 * KWGW], F16, tag=f"g{g % 3}", name=f"g{g % 3}")
        nc.sync.dma_start(g_t[:], g_dram[g])
        xr_t = xr_ts[g % 3]
        for i in range(4):
            c = 4 * g + i
            x_t = x_ts[c % N_XBUF]
            nc.sync.dma_start(x_t[:], x_dram[c])
            o_t = o_ts[c % N_OBUF]
            # kw-outer / tile-inner: 4 consecutive matmuls share one
            # stationary and write 4 different PSUM banks (independent, so
            # the PE can pipeline them back-to-back).
            ps_list = [
                ps_pool.tile([128, w], F32, tag="ps", name=f"ps_t{t}")
                for t in range(NFULL)
            ]
            for idx, kw in enumerate(KWS):
                s = kw - PAD
                w_lo = max(0, -s)
                w_hi = w + min(0, -s)
                lhs = g_t[:128, i * KWGW + kw * GW + PAD :
                          i * KWGW + kw * GW + PAD + MT]
                for t in range(NFULL):
                    rhs = x_t[:128, t * w + w_lo + s : t * w + w_hi + s]
                    nc.tensor.matmul(
                        ps_list[t][:MT, w_lo:w_hi], lhs, rhs,
                        start=(idx == 0), stop=(idx == KW - 1),
                    )
            for t in range(NFULL):
                nc.vector.tensor_copy(
                    o_t[:MT, t * w : (t + 1) * w], ps_list[t][:MT, :]
                )
            nc.scalar.dma_start(y_dram[c], o_t[:, :])
        # Packed runt: 4 channels on diagonal 32x32 PE tiles, kw-outer so the
        # four tiles' matmuls run concurrently.
        nc.sync.dma_start(xr_t[:], xr_dram[g])
        ps_rs = [
            ps_pool.tile([128, w], F32, tag="ps", name=f"ps_r{i}") for i in range(4)
        ]
        for idx, kw in enumerate(KWS):
            s = kw - PAD
            w_lo = max(0, -s)
            w_hi = w + min(0, -s)
            for i in range(4):
                lhs = g_t[32 * i : 32 * i + KR,
                          i * KWGW + kw * GW + PAD + 32 * i :
                          i * KWGW + kw * GW + PAD + 32 * i + MR]
                rhs = xr_t[32 * i : 32 * i + KR, w_lo + s : w_hi + s]
                nc.tensor.matmul(
                    ps_rs[i][32 * i : 32 * i + MR, w_lo:w_hi], lhs, rhs,
                    start=(idx == 0), stop=(idx == KW - 1),
                    tile_position=(32 * i, 32 * i),
                )
        o_r = or_ts[g % 3]
        for i in range(4):
            nc.vector.tensor_copy(
                o_r[32 * i : 32 * i + MR, :], ps_rs[i][32 * i : 32 * i + MR, :]
            )
        nc.scalar.dma_start(yr_dram[g], o_r[:, :])


def build_nc():
    w = W_DIM
    nc = bacc.Bacc(None, target_bir_lowering=False)
    x_dram = nc.dram_tensor("X", [CPC, 128, NFULL * w], F16, kind="ExternalInput")
    xr_dram = nc.dram_tensor("XR", [NG, 128, w], F16, kind="ExternalInput")
    g_dram = nc.dram_tensor("G", [NG, 128, 4 * KWGW], F16, kind="ExternalInput")
    y_dram = nc.dram_tensor("Y", [CPC, 128, NFULL * w], F16, kind="ExternalOutput")
    yr_dram = nc.dram_tensor("YR", [NG, 128, w], F16, kind="ExternalOutput")

    with tile.TileContext(nc) as tc:
        with (
            tc.tile_pool(name="xw", bufs=1) as x_pool,
            tc.tile_pool(name="g", bufs=1) as g_pool,
            tc.tile_pool(name="ps", bufs=8, space="PSUM") as ps_pool,
            tc.tile_pool(name="ob", bufs=1) as o_pool,
        ):
            _emit(nc, x_pool, g_pool, ps_pool, o_pool,
                  x_dram, xr_dram, g_dram, y_dram, yr_dram)

    nc.compile()
    return nc


def build_g(wf):
    """wf: (C, 7, 7) filters -> (C, 128, 7*GW) float16 banded matrices.

    G[c, j, kw*GW + m2] = wf[c, j - m2 + 3, kw] where 0 <= j-m2+3 < 7, else 0.
    """
    c = wf.shape[0]
    g = np.zeros((c, 128, KW, GW), dtype=NP_IN)
    js = np.arange(128)
    for kh in range(KH):
        m2 = js + 3 - kh
        mask = (m2 >= 0) & (m2 < GW)
        g[:, js[mask], :, m2[mask]] = wf[None, :, kh, :].astype(NP_IN)
    return g.reshape(c, 128, KW * GW)


def prep_inputs(X, W):
    """Full-size host prep: windowed X, grouped G, packed runts (fp16)."""
    w = W_DIM
    xp = np.zeros((C, HP, w), dtype=NP_IN)
    xp[:, PAD : PAD + H] = X.astype(NP_IN)

    # [C, 128, 4, w]: partition p of window t = padded row 122t + p
    xwin = np.stack(
        [xp[:, MT * t : MT * t + 128, :] for t in range(NFULL)], axis=2
    ).reshape(C, 128, NFULL * w)

    # [C//4, 128, w]: partitions 32i+r = channel (4g+i) padded row 488+r
    xr = np.zeros((C // 4, 4, 32, w), dtype=NP_IN)
    xr[:, :, :KR, :] = xp[:, NFULL * MT : NFULL * MT + KR, :].reshape(
        C // 4, 4, KR, w
    )
    xr = xr.reshape(C // 4, 128, w)

    g_all = build_g(np.ascontiguousarray(W[:, 0]))  # (C, 128, 875)
    g4 = (
        g_all.reshape(C // 4, 4, 128, KWGW)
        .transpose(0, 2, 1, 3)
        .reshape(C // 4, 128, 4 * KWGW)
    )
    return xwin, xr, g4


def assemble_output(y4, yr):
    """y4: (CPC, 128, 4*w) fp16, yr: (NG, 128, w) fp16 -> (CPC, H, w) f32."""
    w = W_DIM
    main = (
        y4.reshape(CPC, 128, NFULL, w)[:, :MT]
        .transpose(0, 2, 1, 3)
        .reshape(CPC, NFULL * MT, w)
    )
    runt = yr.reshape(NG, 4, 32, w)[:, :, :MR, :].reshape(CPC, MR, w)
    return np.concatenate([main, runt], axis=1).astype(np.float32)


_NC_CACHE = {}


def _get_nc():
    if "nc" not in _NC_CACHE:
        _NC_CACHE["nc"] = build_nc()
    return _NC_CACHE["nc"]


def run(X, W, **spmd_kwargs):
    X = np.asarray(X, dtype=np.float32)
    W = np.asarray(W, dtype=np.float32)
    xwin, xr, g4 = prep_inputs(X, W)

    nc = _get_nc()
    in_maps = []
    for core in range(N_CORES):
        c0 = core * CPC
        g0 = core * NG
        in_maps.append(
            {
                "X": np.ascontiguousarray(xwin[c0 : c0 + CPC]),
                "XR": np.ascontiguousarray(xr[g0 : g0 + NG]),
                "G": np.ascontiguousarray(g4[g0 : g0 + NG]),
            }
        )
    res = run_bass_kernel_spmd(nc, in_maps, core_ids=list(range(N_CORES)),
                               **spmd_kwargs)
    y = np.concatenate(
        [assemble_output(r["Y"], r["YR"]) for r in res.results], axis=0
    )
    return y, res


def kernel(X, W):
    return run(X, W)[0]
